# revision 30
# baseline (speedup 1.0000x reference)
"""Trainium2 Bass kernel for nn_InverseRecurrentLayer.

Reference computation:
    W_inv = inv(W)
    h[t] = inputs[:, t, :] @ R + bias      # [B, U]  (bias folded into h)
    s_{t+1} = tanh(h[t] + s_t @ Wt),  Wt = W if (t//64)%2==1 else W_inv
    output = states [T, B, U]

Shapes: B=64, T=512, F=512, U=1024. fp32 in/out. Data-parallel over
batch: 8 cores x B_loc=8.

Per-core plan (v6 -- W-stationary scan, 497us vs v4's 1937us):
  v4 streamed all of W through the PE as the *moving* tensor every step
  (8192 moving cols/step -> 3.4us/step floor). v6 flips the matmul:
  W chunks [128,128] are *stationary* and the transposed state chunk
  [128, 8] is moving, so a full state update streams only 8kc x 8uo x 8
  = 512 moving cols. All scan operands are bf16 (1 cycle/col at any
  size; fp32 psum accumulate), giving rel_err 1.28e-2 < 2e-2.

  State layout: one tile s[128, 64], col = j*8 + b for u-chunk j, batch
  b; partition = u % 128. This layout is both the matmul moving operand
  (col-slice per kc) and the tanh output -- no transposes anywhere.
  A step is:
    - 1 identity matmul:  psum[128,64]  = I128^T @ h_t[128,64] (start)
    - 64 matmuls:         psum[:, j*8:] += Wc[kc,j]^T @ s_prev[:, kc*8:]
    - 1 ACT tanh:         s_new[128,64](bf16) = tanh(psum)
  s_new lands in a 16-step ring; one DMA per 16 steps stores it (bf16;
  the host converts/untransposes). Steady step = 934ns, fully
  latency-bound: 238 ACT busy + 185 ACT drain + 55 sems + ~250 matmul
  stream (SEQ-bound: bf16 matmuls legalize to Ldweights+Matmult pairs)
  + 173 PE psum drain + 33 sems. Splitting the tanh or the batch makes
  it worse (ACT's 185ns access-latency init is per instruction).

  Phase A (h = x @ R + bias, transposed layout hT[u, (t,j,b)]) runs as
  bf16 matmuls interleaved ~2-3 items/step into the scan's PE idle
  window (~500ns/step), DVE drains psum -> resident h_sb tiles, one
  tile per 32-step chunk to avoid whole-tile hazards with scan reads.

  Hard-won scheduling details:
  - Every 16th tanh recycles a ring buffer; its extra WAR wait would
    legalize into a waitcar NoOp that parks the ACT sequencer (+78ns).
    A dummy ACT write to the fresh ring tile mid-window absorbs it.
  - Output DMAs go on the GPSIMD queue: their waitcar NoOps park the
    issuing sequencer ~14us, which would stall SP's x-staging DMAs.
  - The prologue is DMA-pipe-bound (~3.4MB before step 0 can finish:
    x0/R/x-chunk-0/W_inv). Queue/issue order is tuned so the pipe sees
    step-0 dependencies first; phase-1 W rides the in-order SP queue
    behind them (an independent queue would start its transfers at t=0
    and flood the shared pipe). The final window's output drains in
    8/4/4-slot pieces so the epilogue tail is short.

This environment's walrus encodes at most ONE sync-wait command per
instruction; legalize_waits() hoists extra waits onto InstNoOp carriers,
and the Tile exit barrier is patched to sem-only barriers.
"""
import sys

sys.path.insert(0, "/opt/trn_rl_repo")

import numpy as np
from contextlib import ExitStack

import concourse.bass as bass
import concourse.mybir as mybir
import concourse.tile as tile
from concourse.bass_utils import run_bass_kernel_spmd

# ---------------------------------------------------------------- constants
B, T, F, U = 64, 512, 512, 1024
NCORES = 8
BLOC = B // NCORES          # 8 batch rows per core
KF = F // 128               # 4 f-chunks for the projection
KU = U // 128               # 8 u-chunks
TCH = 32                    # steps per h chunk
RING = 16                   # steps per output ring/DMA
F32 = mybir.dt.float32
F32R = mybir.dt.float32r
BF16 = mybir.dt.bfloat16

# ------------------------------------------------- walrus wait legalization


def _patched_drain_and_barrier(self, tick_clock, wait_clock):
    drain_inst = self.nc.sync.drain()
    wait_clock.add_sem_waits(
        drain_inst.ins, tile.ScopedClock({None: tick_clock.global_clock})
    )
    ow = list(drain_inst.ins.sync_info.on_wait or [])
    if len(ow) > 1:
        drain_inst.ins.sync_info.on_wait = ow[:1]
        for w in ow[1:]:
            d2 = self.nc.sync.drain()
            d2.ins.sync_info = mybir.SyncInfo(on_wait=[w], on_update=[])
    self.nc.all_engine_barrier(sem_only=True)
    popped = self.nc._tile_sem_poison_stack.pop()
    assert popped is self._sem_poison
    self.nc.clear_and_free_semaphores(list(self.sems.allocated().values()))
    self.nc.all_engine_barrier(sem_only=True)


tile.TileContext._drain_and_barrier = _patched_drain_and_barrier


def legalize_waits(nc):
    """Split multi-wait instructions: keep 1 wait, hoist the rest onto
    InstNoOp carriers inserted just before, on the same engine."""
    n = 0
    for fn in nc.m.functions:
        for blk in fn.blocks:
            out = []
            for inst in blk.instructions:
                si = inst.sync_info
                if si is not None and si.on_wait and len(si.on_wait) > 1:
                    waits = list(si.on_wait)
                    for w in waits[:-1]:
                        n += 1
                        nop = mybir.InstNoOp(
                            name=f"waitcar-{n}-{inst.name}",
                            engine=inst.engine,
                            ins=[],
                            outs=[],
                            sync_info=mybir.SyncInfo(on_wait=[w], on_update=[]),
                        )
                        nc.register_instruction(nop)
                        out.append(nop)
                    si.on_wait = waits[-1:]
                out.append(inst)
            blk.instructions[:] = out
    return n


# ------------------------------------------------------------ device kernel


def build_kernel(t_steps=T, with_bias=False):
    assert t_steps % TCH == 0 and t_steps % RING == 0
    nc = bass.Bass("TRN2", target_bir_lowering=False, debug=False)
    tb = BLOC * t_steps
    n_ch = (t_steps + TCH - 1) // TCH       # h chunks (64 steps each)

    xT_d = nc.dram_tensor("xT", [F, tb], BF16, kind="ExternalInput").ap()
    R_d = nc.dram_tensor("R", [F, U], BF16, kind="ExternalInput").ap()
    W2_d = nc.dram_tensor("W2", [2, U, U], BF16, kind="ExternalInput").ap()
    x0_d = nc.dram_tensor("x0b", [128, KU * BLOC], BF16,
                          kind="ExternalInput").ap()
    id_d = nc.dram_tensor("id128", [128, 128], BF16, kind="ExternalInput").ap()
    if with_bias:
        bias_d = nc.dram_tensor("biasT", [1, U], BF16,
                                kind="ExternalInput").ap()
    out_d = nc.dram_tensor("out", [128, t_steps * KU * BLOC], BF16,
                           kind="ExternalOutput").ap()

    SW = KU * BLOC            # 64: state width (j*8 + b)

    with tile.TileContext(nc) as tc, ExitStack() as ctx:
        const = ctx.enter_context(tc.tile_pool(name="const", bufs=1))
        # resident tensors. W tiles are split per (phase, kc) so the scan's
        # first steps only wait on the W_inv DMAs.
        w0_sb = [const.tile([128, 4 * U], BF16, tag=f"w0_{h}", name=f"w0_{h}")
                 for h in range(2)]
        w1_sb = [const.tile([128, U], BF16, tag=f"w1_{k}", name=f"w1_{k}")
                 for k in range(KU)]

        def wslice(p, k, j):
            if p == 0:
                return w0_sb[k // 4][:, (k % 4) * U + j * 128:
                                     (k % 4) * U + (j + 1) * 128]
            return w1_sb[k][:, j * 128:(j + 1) * 128]
        ra_sb = [const.tile([128, U], BF16, tag=f"ra{k}", name=f"ra{k}") for k in range(KF)]
        h_sb = [const.tile([128, TCH * SW], BF16, tag=f"h{c}", name=f"h{c}")
                for c in range(n_ch)]
        x0_sb = const.tile([128, SW], BF16, tag="x0")
        id_sb = const.tile([128, 128], BF16, tag="id")
        if with_bias:
            bias_sb = const.tile([1, U], BF16, tag="bias")
            ones_sb = const.tile([1, 512], BF16, tag="ones")

        # prologue DMAs: step-0 needs x0 (PE warmup), R + x chunk 0 (the
        # h projection) and all of W_inv. W_inv goes as two fat DMAs (one
        # per HWDGE queue) behind the small tensors; the phase-1 W tiles
        # (not needed until step 64, ~70us in) are issued mid-scan from the
        # phase-A item stream so they don't contend for the DMA pipe here.
        W2v = W2_d.rearrange("q (k p) u -> q p k u", p=128)
        nc.scalar.dma_start(x0_sb[:, :], x0_d[:, :])
        for k in (0, 1):
            nc.scalar.dma_start(ra_sb[k][:, :], R_d[k * 128:(k + 1) * 128, :])
        sp_prologue = [
            lambda: nc.sync.dma_start(ra_sb[2][:, :], R_d[256:384, :]),
            lambda: nc.sync.dma_start(ra_sb[3][:, :], R_d[384:512, :]),
            lambda: nc.sync.dma_start(id_sb[:, :], id_d[:, :]),
            lambda: nc.sync.dma_start(
                w0_sb[0].rearrange("p (k u) -> p k u", u=U), W2v[0, :, 0:4, :]),
        ]
        nc.scalar.dma_start(
            w0_sb[1].rearrange("p (k u) -> p k u", u=U), W2v[0, :, 4:8, :])
        if with_bias:
            nc.scalar.dma_start(bias_sb[:, :], bias_d[:, :])
            nc.vector.memset(ones_sb[:, :], 1.0)

        # ---------------- phase A: hT = (x @ R + bias)^T -> bf16 SBUF
        # h_sb[c][p, tl*SW + j*8 + b] = h[t = c*TCH + tl, u = j*128 + p, b]
        CW = TCH * BLOC          # 256 moving cols per pa matmul
        xpool = ctx.enter_context(tc.tile_pool(name="xstage", bufs=2))
        papool = ctx.enter_context(
            tc.tile_pool(name="psum_pa", bufs=2, space="PSUM")
        )
        pa_state = {}

        xT_v = xT_d.rearrange("(k p) t -> p k t", k=KF)

        def pa_dma_item(c):
            def run():
                xa = xpool.tile([128, KF, CW], BF16, tag="xa", name="xa")
                nc.sync.dma_start(xa[:, :, :], xT_v[:, :, c * CW:(c + 1) * CW])
                pa_state[c] = xa
            return run

        def pa_mm_item(c, j, k):
            def run():
                if k == 0:
                    pa_state["ps"] = papool.tile([128, CW], F32, tag="psA", name="psA")
                last = (k == KF - 1) and not with_bias
                nc.tensor.matmul(
                    pa_state["ps"][:, :],
                    ra_sb[k][:, j * 128:(j + 1) * 128],
                    pa_state[c][:, k, :],
                    start=(k == 0),
                    stop=last,
                )
                if with_bias and k == KF - 1:
                    nc.tensor.matmul(
                        pa_state["ps"][:, :],
                        bias_sb[0:1, j * 128:(j + 1) * 128],
                        ones_sb[0:1, :CW],
                        start=False,
                        stop=True,
                    )
            return run

        def pa_copy_item(c, j):
            def run():
                # psum [128, CW] = [tl, b] -> h_sb[c] cols tl*64 + j*8 + b
                dst = h_sb[c].rearrange("p (t w) -> p t w", w=SW)[:, :, j * 8:(j + 1) * 8]
                src = pa_state["ps"].rearrange("p (t b) -> p t b", b=8)[:, :, :]
                nc.vector.tensor_copy(dst, src)
            return run

        pa_items = []
        for c in range(n_ch):
            if c == 0:
                pa_items.append(pa_dma_item(0))
            for j in range(KU):
                # prefetch next chunk's x at the midpoint of this chunk
                if j == KU // 2 and c + 1 < n_ch:
                    pa_items.append(pa_dma_item(c + 1))
                for k in range(KF):
                    pa_items.append(pa_mm_item(c, j, k))
                pa_items.append(pa_copy_item(c, j))
        pa_pos = [0]

        def pa_pull(limit):
            done = 0
            while done < limit and pa_pos[0] < len(pa_items):
                pa_items[pa_pos[0]]()
                pa_pos[0] += 1
                done += 1

        # xa0 fat DMA first on SP, then the remaining W_inv tiles
        pa_pull(1)
        for f in sp_prologue:
            f()

        # bootstrap: chunk 0 only (the scan starts after ~32 pa matmuls)
        boot_items = KU * (KF + 1) + 1
        pa_pull(boot_items)

        # phase-1 W tiles: on SP *after* the prologue/xa DMAs. SP is
        # in-order, so these 8 transfers hit the DMA pipe only once the
        # step-0 dependencies are through (they're not needed until ~70us).
        for k in range(KU):
            nc.sync.dma_start(w1_sb[k][:, :], W2_d[1, k * 128:(k + 1) * 128, :])


        # ---------------- phase B: the scan
        mmpool = ctx.enter_context(
            tc.tile_pool(name="psum_mm", bufs=2, space="PSUM")
        )
        rpool = ctx.enter_context(tc.tile_pool(name="ring", bufs=2))

        ring = None
        ring_next = [None]
        prev_tile, prev_off = x0_sb, 0
        for t in range(t_steps):
            c, tl = t // TCH, t % TCH
            p = 1 if (t // 64) % 2 == 1 else 0      # INVERT_INDEX = 64
            if t % RING == 0:
                if ring_next[0] is not None:
                    ring = ring_next[0]
                    ring_next[0] = None
                else:
                    ring = rpool.tile([128, RING * SW], BF16, tag="ring",
                                      name="ring")
            ps = mmpool.tile([128, SW], F32, tag="ps", name="ps")
            # h first: no dependency on the previous state, so it fills the
            # PE idle window while tanh(t-1) is still in flight.
            nc.tensor.matmul(
                ps[:, :],
                id_sb[:, :],
                h_sb[c][:, tl * SW:(tl + 1) * SW],
                start=True,
                stop=False,
            )
            for j in range(KU):
                for k in range(KU):
                    nc.tensor.matmul(
                        ps[:, j * 8:(j + 1) * 8],
                        wslice(p, k, j),
                        prev_tile[:, prev_off + k * 8:prev_off + (k + 1) * 8],
                        start=False,
                        stop=(j == KU - 1 and k == KU - 1),
                    )
            # phase-A filler work rides in the PE bubble created by the
            # tanh wait (emitted after this step's mms, before the next's).
            # Steps 0-7: no pulls (chunk-1 items would park the PE queue on
            # the late-landing xa1 DMA); catch up afterwards.
            pa_pull(0 if t < 2 else (3 if t < 24 else 2))
            so = (t % RING) * SW
            nc.scalar.activation(
                ring[:, so:so + SW],
                ps[:, :],
                mybir.ActivationFunctionType.Tanh,
            )
            prev_tile, prev_off = ring, so
            if t % RING == RING // 2 and t + RING < t_steps:
                # pre-allocate the next ring and absorb its buffer-recycle
                # WAR wait into an off-path ACT dummy write: without this
                # the first tanh of each window carries 2 sem waits and the
                # legalized waitcar NoOp parks the ACT sequencer (+78ns).
                rn = rpool.tile([128, RING * SW], BF16, tag="ring",
                                name="ring")
                nc.scalar.activation(
                    rn[:, 0:8], x0_sb[:, 0:8],
                    mybir.ActivationFunctionType.Copy,
                )
                ring_next[0] = rn
            if t >= t_steps - RING and (
                t == t_steps - 9 or t == t_steps - 5 or t == t_steps - 1
            ) and t % RING != RING - 1 or (
                t == t_steps - 1 and t % RING == RING - 1
            ):
                # drain the final window in shrinking pieces (8/4/4 slots)
                # so the epilogue tail only waits on a 4-slot DMA (on SP:
                # idle by now and HWDGE issues faster than SWDGE)
                w0 = t_steps - RING              # final window start
                lo = {t_steps - 9: 0, t_steps - 5: 8, t_steps - 1: 12}[t]
                hi = {t_steps - 9: 8, t_steps - 5: 12, t_steps - 1: 16}[t]
                nc.sync.dma_start(
                    out_d[:, (w0 + lo) * SW:(w0 + hi) * SW],
                    ring[:, lo * SW:hi * SW],
                )
            elif t % RING == RING - 1:
                nc.gpsimd.dma_start(
                    out_d[:, (t - RING + 1) * SW:(t + 1) * SW], ring[:, :]
                )
        pa_pull(len(pa_items))      # drain any leftovers (shouldn't exist)

    legalize_waits(nc)
    return nc


# -------------------------------------------------------------- host driver
_CACHE = {}


def _get_nc(t_steps, with_bias=False):
    key = (t_steps, with_bias)
    if key not in _CACHE:
        _CACHE[key] = build_kernel(t_steps, with_bias)
    return _CACHE[key]


def kernel(inputs, R, W, bias, x0, t_steps=None, n_cores=NCORES, trace=False,
           trace_kw=None):
    import ml_dtypes
    bf16 = ml_dtypes.bfloat16

    t_steps = t_steps or inputs.shape[1]
    inputs = np.ascontiguousarray(inputs, dtype=np.float32)
    R = np.asarray(R, dtype=np.float32)
    W = np.asarray(W, dtype=np.float32)
    bias = np.asarray(bias, dtype=np.float32)
    x0 = np.asarray(x0, dtype=np.float32)

    W_inv = np.linalg.inv(W)
    W2 = np.stack([W_inv, W]).astype(bf16)              # phase 0 = W_inv
    Rb = R.astype(bf16)
    # x0b[p, j*8+b] = x0[j*128+p]
    x0b = np.repeat(
        x0.reshape(KU, 128).T[:, :, None], BLOC, axis=2
    ).reshape(128, KU * BLOC).astype(bf16)
    id128 = np.eye(128, dtype=np.float32).astype(bf16)
    with_bias = bool(np.any(bias))
    biasT = None
    if with_bias:
        biasT = np.ascontiguousarray(bias.reshape(1, U)).astype(bf16)

    in_maps = []
    for c in range(n_cores):
        xc = inputs[c * BLOC:(c + 1) * BLOC, :t_steps, :]   # [BLOC, t, F]
        xT = np.ascontiguousarray(
            xc.transpose(2, 1, 0).reshape(F, BLOC * t_steps)
        ).astype(bf16)
        m = {"xT": xT, "R": Rb, "W2": W2, "x0b": x0b, "id128": id128}
        if with_bias:
            m["biasT"] = biasT
        in_maps.append(m)

    nc = _get_nc(t_steps, with_bias)
    try:
        res = run_bass_kernel_spmd(
            nc, in_maps, core_ids=list(range(n_cores)), trace=trace,
            **(trace_kw or {}),
        )
    except Exception:
        # transient device wedges (NRT_EXEC_UNIT_UNRECOVERABLE) usually
        # clear on a retry
        res = run_bass_kernel_spmd(
            nc, in_maps, core_ids=list(range(n_cores)), trace=trace,
            **(trace_kw or {}),
        )
    kernel.last_result = res
    kernel.last_nc = nc
    # assemble [T, B, U]: per-core out is [128, t*64] bf16 transposed state
    full = np.empty((t_steps, n_cores * BLOC, U), np.float32)
    for c in range(n_cores):
        arr = np.asarray(res.results[c]["out"])          # [128, t*64] bf16
        full[:, c * BLOC:(c + 1) * BLOC, :] = (
            arr.reshape(128, t_steps, KU, BLOC)
            .transpose(1, 3, 2, 0)
            .reshape(t_steps, BLOC, U)
            .astype(np.float32)
        )
    return full


# revision 36
# speedup vs baseline: 1.0014x; 1.0014x over previous
"""Trainium2 Bass kernel for nn_InverseRecurrentLayer.

Reference computation:
    W_inv = inv(W)
    h[t] = inputs[:, t, :] @ R + bias      # [B, U]  (bias folded into h)
    s_{t+1} = tanh(h[t] + s_t @ Wt),  Wt = W if (t//64)%2==1 else W_inv
    output = states [T, B, U]

Shapes: B=64, T=512, F=512, U=1024. fp32 in/out. Data-parallel over
batch: 8 cores x B_loc=8.

Per-core plan (v6 -- W-stationary scan, 497us vs v4's 1937us):
  v4 streamed all of W through the PE as the *moving* tensor every step
  (8192 moving cols/step -> 3.4us/step floor). v6 flips the matmul:
  W chunks [128,128] are *stationary* and the transposed state chunk
  [128, 8] is moving, so a full state update streams only 8kc x 8uo x 8
  = 512 moving cols. All scan operands are bf16 (1 cycle/col at any
  size; fp32 psum accumulate), giving rel_err 1.28e-2 < 2e-2.

  State layout: one tile s[128, 64], col = j*8 + b for u-chunk j, batch
  b; partition = u % 128. This layout is both the matmul moving operand
  (col-slice per kc) and the tanh output -- no transposes anywhere.
  A step is:
    - 1 identity matmul:  psum[128,64]  = I128^T @ h_t[128,64] (start)
    - 64 matmuls:         psum[:, j*8:] += Wc[kc,j]^T @ s_prev[:, kc*8:]
    - 1 ACT tanh:         s_new[128,64](bf16) = tanh(psum)
  s_new lands in a 16-step ring; one DMA per 16 steps stores it (bf16;
  the host converts/untransposes). Steady step = 934ns, fully
  latency-bound: 238 ACT busy + 185 ACT drain + 55 sems + ~250 matmul
  stream (SEQ-bound: bf16 matmuls legalize to Ldweights+Matmult pairs)
  + 173 PE psum drain + 33 sems. Splitting the tanh or the batch makes
  it worse (ACT's 185ns access-latency init is per instruction).

  Phase A (h = x @ R + bias, transposed layout hT[u, (t,j,b)]) runs as
  bf16 matmuls interleaved ~2-3 items/step into the scan's PE idle
  window (~500ns/step), DVE drains psum -> resident h_sb tiles, one
  tile per 32-step chunk to avoid whole-tile hazards with scan reads.

  Hard-won scheduling details:
  - Every 16th tanh recycles a ring buffer; its extra WAR wait would
    legalize into a waitcar NoOp that parks the ACT sequencer (+78ns).
    A dummy ACT write to the fresh ring tile mid-window absorbs it.
  - Output DMAs go on the GPSIMD queue: their waitcar NoOps park the
    issuing sequencer ~14us, which would stall SP's x-staging DMAs.
  - The prologue is DMA-pipe-bound (~3.4MB before step 0 can finish:
    x0/R/x-chunk-0/W_inv). Queue/issue order is tuned so the pipe sees
    step-0 dependencies first; phase-1 W rides the in-order SP queue
    behind them (an independent queue would start its transfers at t=0
    and flood the shared pipe). The final window's output drains in
    8/4/4-slot pieces so the epilogue tail is short.

This environment's walrus encodes at most ONE sync-wait command per
instruction; legalize_waits() hoists extra waits onto InstNoOp carriers,
and the Tile exit barrier is patched to sem-only barriers.
"""
import sys

sys.path.insert(0, "/opt/trn_rl_repo")

import numpy as np
from contextlib import ExitStack

import concourse.bass as bass
import concourse.mybir as mybir
import concourse.tile as tile
from concourse.bass_utils import run_bass_kernel_spmd

# ---------------------------------------------------------------- constants
B, T, F, U = 64, 512, 512, 1024
NCORES = 8
BLOC = B // NCORES          # 8 batch rows per core
KF = F // 128               # 4 f-chunks for the projection
KU = U // 128               # 8 u-chunks
TCH = 32                    # steps per h chunk
RING = 16                   # steps per output ring/DMA
F32 = mybir.dt.float32
F32R = mybir.dt.float32r
BF16 = mybir.dt.bfloat16

# ------------------------------------------------- walrus wait legalization


def _patched_drain_and_barrier(self, tick_clock, wait_clock):
    drain_inst = self.nc.sync.drain()
    wait_clock.add_sem_waits(
        drain_inst.ins, tile.ScopedClock({None: tick_clock.global_clock})
    )
    ow = list(drain_inst.ins.sync_info.on_wait or [])
    if len(ow) > 1:
        drain_inst.ins.sync_info.on_wait = ow[:1]
        for w in ow[1:]:
            d2 = self.nc.sync.drain()
            d2.ins.sync_info = mybir.SyncInfo(on_wait=[w], on_update=[])
    self.nc.all_engine_barrier(sem_only=True)
    popped = self.nc._tile_sem_poison_stack.pop()
    assert popped is self._sem_poison
    self.nc.clear_and_free_semaphores(list(self.sems.allocated().values()))
    self.nc.all_engine_barrier(sem_only=True)


tile.TileContext._drain_and_barrier = _patched_drain_and_barrier


def legalize_waits(nc):
    """Split multi-wait instructions: keep 1 wait, hoist the rest onto
    InstNoOp carriers inserted just before, on the same engine."""
    n = 0
    for fn in nc.m.functions:
        for blk in fn.blocks:
            out = []
            for inst in blk.instructions:
                si = inst.sync_info
                if si is not None and si.on_wait and len(si.on_wait) > 1:
                    waits = list(si.on_wait)
                    for w in waits[:-1]:
                        n += 1
                        nop = mybir.InstNoOp(
                            name=f"waitcar-{n}-{inst.name}",
                            engine=inst.engine,
                            ins=[],
                            outs=[],
                            sync_info=mybir.SyncInfo(on_wait=[w], on_update=[]),
                        )
                        nc.register_instruction(nop)
                        out.append(nop)
                    si.on_wait = waits[-1:]
                out.append(inst)
            blk.instructions[:] = out
    return n


# ------------------------------------------------------------ device kernel


def build_kernel(t_steps=T, with_bias=False):
    assert t_steps % TCH == 0 and t_steps % RING == 0
    nc = bass.Bass("TRN2", target_bir_lowering=False, debug=False)
    tb = BLOC * t_steps
    n_ch = (t_steps + TCH - 1) // TCH       # h chunks (64 steps each)

    xT_d = nc.dram_tensor("xT", [F, tb], BF16, kind="ExternalInput").ap()
    R_d = nc.dram_tensor("R", [F, U], BF16, kind="ExternalInput").ap()
    W2_d = nc.dram_tensor("W2", [2, U, U], BF16, kind="ExternalInput").ap()
    x0_d = nc.dram_tensor("x0b", [128, KU * BLOC], BF16,
                          kind="ExternalInput").ap()
    id_d = nc.dram_tensor("id128", [128, 128], BF16, kind="ExternalInput").ap()
    if with_bias:
        bias_d = nc.dram_tensor("biasT", [1, U], BF16,
                                kind="ExternalInput").ap()
    out_d = nc.dram_tensor("out", [128, t_steps * KU * BLOC], BF16,
                           kind="ExternalOutput").ap()

    SW = KU * BLOC            # 64: state width (j*8 + b)

    with tile.TileContext(nc) as tc, ExitStack() as ctx:
        const = ctx.enter_context(tc.tile_pool(name="const", bufs=1))
        # resident tensors. W tiles are split per (phase, kc) so the scan's
        # first steps only wait on the W_inv DMAs.
        w0_sb = [const.tile([128, 4 * U], BF16, tag=f"w0_{h}", name=f"w0_{h}")
                 for h in range(2)]
        w1_sb = [const.tile([128, U], BF16, tag=f"w1_{k}", name=f"w1_{k}")
                 for k in range(KU)]

        def wslice(p, k, j):
            if p == 0:
                return w0_sb[k // 4][:, (k % 4) * U + j * 128:
                                     (k % 4) * U + (j + 1) * 128]
            return w1_sb[k][:, j * 128:(j + 1) * 128]
        ra_sb = [const.tile([128, U], BF16, tag=f"ra{k}", name=f"ra{k}") for k in range(KF)]
        h_sb = [const.tile([128, TCH * SW], BF16, tag=f"h{c}", name=f"h{c}")
                for c in range(n_ch)]
        x0_sb = const.tile([128, SW], BF16, tag="x0")
        id_sb = const.tile([128, 128], BF16, tag="id")
        if with_bias:
            bias_sb = const.tile([1, U], BF16, tag="bias")
            ones_sb = const.tile([1, 512], BF16, tag="ones")

        # prologue DMAs: step-0 needs x0 (PE warmup), R + x chunk 0 (the
        # h projection) and all of W_inv. W_inv goes as two fat DMAs (one
        # per HWDGE queue) behind the small tensors; the phase-1 W tiles
        # (not needed until step 64, ~70us in) are issued mid-scan from the
        # phase-A item stream so they don't contend for the DMA pipe here.
        W2v = W2_d.rearrange("q (k p) u -> q p k u", p=128)
        nc.scalar.dma_start(x0_sb[:, :], x0_d[:, :])
        for k in (0, 1):
            nc.scalar.dma_start(ra_sb[k][:, :], R_d[k * 128:(k + 1) * 128, :])
        sp_prologue = [
            lambda: nc.sync.dma_start(ra_sb[2][:, :], R_d[256:384, :]),
            lambda: nc.sync.dma_start(ra_sb[3][:, :], R_d[384:512, :]),
            lambda: nc.sync.dma_start(id_sb[:, :], id_d[:, :]),
            lambda: nc.sync.dma_start(
                w0_sb[0].rearrange("p (k u) -> p k u", u=U), W2v[0, :, 0:4, :]),
        ]
        nc.scalar.dma_start(
            w0_sb[1].rearrange("p (k u) -> p k u", u=U), W2v[0, :, 4:8, :])
        if with_bias:
            nc.scalar.dma_start(bias_sb[:, :], bias_d[:, :])
            nc.vector.memset(ones_sb[:, :], 1.0)

        # ---------------- phase A: hT = (x @ R + bias)^T -> bf16 SBUF
        # h_sb[c][p, tl*SW + j*8 + b] = h[t = c*TCH + tl, u = j*128 + p, b]
        CW = TCH * BLOC          # 256 moving cols per pa matmul
        xpool = ctx.enter_context(tc.tile_pool(name="xstage", bufs=2))
        papool = ctx.enter_context(
            tc.tile_pool(name="psum_pa", bufs=4, space="PSUM")
        )
        pa_state = {}

        xT_v = xT_d.rearrange("(k p) t -> p k t", k=KF)

        def pa_dma_item(c):
            def run():
                xa = xpool.tile([128, KF, CW], BF16, tag="xa", name="xa")
                nc.sync.dma_start(xa[:, :, :], xT_v[:, :, c * CW:(c + 1) * CW])
                pa_state[c] = xa
            return run

        def pa_mm_item(c, j, k):
            def run():
                if k == 0:
                    pa_state["ps"] = papool.tile([128, CW], F32, tag="psA", name="psA")
                last = (k == KF - 1) and not with_bias
                nc.tensor.matmul(
                    pa_state["ps"][:, :],
                    ra_sb[k][:, j * 128:(j + 1) * 128],
                    pa_state[c][:, k, :],
                    start=(k == 0),
                    stop=last,
                )
                if with_bias and k == KF - 1:
                    nc.tensor.matmul(
                        pa_state["ps"][:, :],
                        bias_sb[0:1, j * 128:(j + 1) * 128],
                        ones_sb[0:1, :CW],
                        start=False,
                        stop=True,
                    )
            return run

        def pa_copy_item(c, j):
            def run():
                # psum [128, CW] = [tl, b] -> h_sb[c] cols tl*64 + j*8 + b
                dst = h_sb[c].rearrange("p (t w) -> p t w", w=SW)[:, :, j * 8:(j + 1) * 8]
                src = pa_state["ps"].rearrange("p (t b) -> p t b", b=8)[:, :, :]
                nc.vector.tensor_copy(dst, src)
            return run

        pa_items = []
        for c in range(n_ch):
            if c == 0:
                pa_items.append(pa_dma_item(0))
            for j in range(KU):
                # prefetch next chunk's x at the midpoint of this chunk
                if j == KU // 2 and c + 1 < n_ch:
                    pa_items.append(pa_dma_item(c + 1))
                for k in range(KF):
                    pa_items.append(pa_mm_item(c, j, k))
                pa_items.append(pa_copy_item(c, j))
        pa_pos = [0]

        def pa_pull(limit):
            done = 0
            while done < limit and pa_pos[0] < len(pa_items):
                pa_items[pa_pos[0]]()
                pa_pos[0] += 1
                done += 1

        # xa0 fat DMA first on SP, then the remaining W_inv tiles
        pa_pull(1)
        for f in sp_prologue:
            f()

        # bootstrap: chunk 0 only (the scan starts after ~32 pa matmuls)
        boot_items = KU * (KF + 1) + 1
        pa_pull(boot_items)

        # phase-1 W tiles: on SP *after* the prologue/xa DMAs. SP is
        # in-order, so these 8 transfers hit the DMA pipe only once the
        # step-0 dependencies are through (they're not needed until ~70us).
        for k in range(KU):
            nc.sync.dma_start(w1_sb[k][:, :], W2_d[1, k * 128:(k + 1) * 128, :])


        # ---------------- phase B: the scan
        mmpool = ctx.enter_context(
            tc.tile_pool(name="psum_mm", bufs=2, space="PSUM")
        )
        rpool = ctx.enter_context(tc.tile_pool(name="ring", bufs=2))

        ring = None
        ring_next = [None]
        prev_tile, prev_off = x0_sb, 0
        for t in range(t_steps):
            c, tl = t // TCH, t % TCH
            p = 1 if (t // 64) % 2 == 1 else 0      # INVERT_INDEX = 64
            if t % RING == 0:
                if ring_next[0] is not None:
                    ring = ring_next[0]
                    ring_next[0] = None
                else:
                    ring = rpool.tile([128, RING * SW], BF16, tag="ring",
                                      name="ring")
            ps = mmpool.tile([128, SW], F32, tag="ps", name="ps")
            # h first: no dependency on the previous state, so it fills the
            # PE idle window while tanh(t-1) is still in flight.
            nc.tensor.matmul(
                ps[:, :],
                id_sb[:, :],
                h_sb[c][:, tl * SW:(tl + 1) * SW],
                start=True,
                stop=False,
            )
            for j in range(KU):
                for k in range(KU):
                    nc.tensor.matmul(
                        ps[:, j * 8:(j + 1) * 8],
                        wslice(p, k, j),
                        prev_tile[:, prev_off + k * 8:prev_off + (k + 1) * 8],
                        start=False,
                        stop=(j == KU - 1 and k == KU - 1),
                    )
            # phase-A filler work rides in the PE bubble created by the
            # tanh wait (emitted after this step's mms, before the next's).
            # Steps 0-7: no pulls (chunk-1 items would park the PE queue on
            # the late-landing xa1 DMA); catch up afterwards.
            pa_pull(0 if t < 4 else (3 if t < 48 else 2))
            so = (t % RING) * SW
            nc.scalar.activation(
                ring[:, so:so + SW],
                ps[:, :],
                mybir.ActivationFunctionType.Tanh,
            )
            prev_tile, prev_off = ring, so
            if t % RING == RING // 2 and t + RING < t_steps:
                # pre-allocate the next ring and absorb its buffer-recycle
                # WAR wait into an off-path ACT dummy write: without this
                # the first tanh of each window carries 2 sem waits and the
                # legalized waitcar NoOp parks the ACT sequencer (+78ns).
                rn = rpool.tile([128, RING * SW], BF16, tag="ring",
                                name="ring")
                nc.scalar.activation(
                    rn[:, 0:8], x0_sb[:, 0:8],
                    mybir.ActivationFunctionType.Copy,
                )
                ring_next[0] = rn
            if t >= t_steps - RING and (
                t == t_steps - 9 or t == t_steps - 5 or t == t_steps - 1
            ) and t % RING != RING - 1 or (
                t == t_steps - 1 and t % RING == RING - 1
            ):
                # drain the final window in shrinking pieces (8/4/4 slots)
                # so the epilogue tail only waits on a 4-slot DMA (on SP:
                # idle by now and HWDGE issues faster than SWDGE)
                w0 = t_steps - RING              # final window start
                lo = {t_steps - 9: 0, t_steps - 5: 8, t_steps - 1: 12}[t]
                hi = {t_steps - 9: 8, t_steps - 5: 12, t_steps - 1: 16}[t]
                nc.sync.dma_start(
                    out_d[:, (w0 + lo) * SW:(w0 + hi) * SW],
                    ring[:, lo * SW:hi * SW],
                )
            elif t % RING == RING - 1:
                nc.gpsimd.dma_start(
                    out_d[:, (t - RING + 1) * SW:(t + 1) * SW], ring[:, :]
                )
        pa_pull(len(pa_items))      # drain any leftovers (shouldn't exist)

    legalize_waits(nc)
    return nc


# -------------------------------------------------------------- host driver
_CACHE = {}


def _get_nc(t_steps, with_bias=False):
    key = (t_steps, with_bias)
    if key not in _CACHE:
        _CACHE[key] = build_kernel(t_steps, with_bias)
    return _CACHE[key]


def kernel(inputs, R, W, bias, x0, t_steps=None, n_cores=NCORES, trace=False,
           trace_kw=None):
    import ml_dtypes
    bf16 = ml_dtypes.bfloat16

    t_steps = t_steps or inputs.shape[1]
    inputs = np.ascontiguousarray(inputs, dtype=np.float32)
    R = np.asarray(R, dtype=np.float32)
    W = np.asarray(W, dtype=np.float32)
    bias = np.asarray(bias, dtype=np.float32)
    x0 = np.asarray(x0, dtype=np.float32)

    W_inv = np.linalg.inv(W)
    W2 = np.stack([W_inv, W]).astype(bf16)              # phase 0 = W_inv
    Rb = R.astype(bf16)
    # x0b[p, j*8+b] = x0[j*128+p]
    x0b = np.repeat(
        x0.reshape(KU, 128).T[:, :, None], BLOC, axis=2
    ).reshape(128, KU * BLOC).astype(bf16)
    id128 = np.eye(128, dtype=np.float32).astype(bf16)
    with_bias = bool(np.any(bias))
    biasT = None
    if with_bias:
        biasT = np.ascontiguousarray(bias.reshape(1, U)).astype(bf16)

    in_maps = []
    for c in range(n_cores):
        xc = inputs[c * BLOC:(c + 1) * BLOC, :t_steps, :]   # [BLOC, t, F]
        xT = np.ascontiguousarray(
            xc.transpose(2, 1, 0).reshape(F, BLOC * t_steps)
        ).astype(bf16)
        m = {"xT": xT, "R": Rb, "W2": W2, "x0b": x0b, "id128": id128}
        if with_bias:
            m["biasT"] = biasT
        in_maps.append(m)

    nc = _get_nc(t_steps, with_bias)
    try:
        res = run_bass_kernel_spmd(
            nc, in_maps, core_ids=list(range(n_cores)), trace=trace,
            **(trace_kw or {}),
        )
    except Exception:
        # transient device wedges (NRT_EXEC_UNIT_UNRECOVERABLE) usually
        # clear on a retry
        res = run_bass_kernel_spmd(
            nc, in_maps, core_ids=list(range(n_cores)), trace=trace,
            **(trace_kw or {}),
        )
    kernel.last_result = res
    kernel.last_nc = nc
    # assemble [T, B, U]: per-core out is [128, t*64] bf16 transposed state
    full = np.empty((t_steps, n_cores * BLOC, U), np.float32)
    for c in range(n_cores):
        arr = np.asarray(res.results[c]["out"])          # [128, t*64] bf16
        full[:, c * BLOC:(c + 1) * BLOC, :] = (
            arr.reshape(128, t_steps, KU, BLOC)
            .transpose(1, 3, 2, 0)
            .reshape(t_steps, BLOC, U)
            .astype(np.float32)
        )
    return full
